# revision 1
# baseline (speedup 1.0000x reference)
"""Causal single-head attention (S=4096, D=1024, fp32) on 8 TRN2 NeuronCores.

v6 (pair-split proj + chunked pair-AllGather + SBUF-accumulated A@V) with the
serialization fixed: attention quarters are emitted INSIDE the projection
stream (attn t0 between proj q2 and q3, the rest after), so the PE consumes
gathered chunks as they land instead of finishing all projections first.
Projection accumulation and score matmuls share one PSUM pool (4 banks) so
the total PSUM stays at 8: 4 shared + 3 A@V scratch + 1 packed sums.
"""

import numpy as np
import ml_dtypes

import concourse.bacc as bacc
import concourse.tile as tile
from concourse import mybir
from concourse.bass_utils import run_bass_kernel_spmd

S = 4096
D = 1024
NCORES = 8
ROWS = 512
P = 128
DC = 8
OT = 8
HALF = 2048
NQT = 4
NJT = 32
BF = mybir.dt.bfloat16
F32 = mybir.dt.float32
EXP = mybir.ActivationFunctionType.Exp
PAIRS = [[0, 1], [2, 3], [4, 5], [6, 7]]

bf16 = ml_dtypes.bfloat16


def build_nc():
    nc = bacc.Bacc(None, target_bir_lowering=False, debug=False)

    xq = nc.declare_dram_parameter("xqt", [D, ROWS], BF, isOutput=False)
    xk = nc.declare_dram_parameter("xkh", [D, HALF], BF, isOutput=False)
    xv = nc.declare_dram_parameter("xvh", [D, HALF], BF, isOutput=False)
    wq = nc.declare_dram_parameter("wqt", [D, D], BF, isOutput=False)
    wk = nc.declare_dram_parameter("wkt", [D, D], BF, isOutput=False)
    wv = nc.declare_dram_parameter("wvt", [D, D], BF, isOutput=False)
    msk = nc.declare_dram_parameter("mask01", [NJT, P, ROWS], BF, isOutput=False)
    out = nc.declare_dram_parameter("out", [ROWS, D], F32, isOutput=True)

    kvin = [nc.dram_tensor(f"kvin{t}", [P, 16, 512], BF) for t in range(NQT)]
    kvout = [nc.dram_tensor(f"kvout{t}", [2 * P, 16, 512], BF) for t in range(NQT)]


    with tile.TileContext(nc) as tc:
        with (
            tc.tile_pool(name="persist", bufs=1) as persist,
            tc.tile_pool(name="proj", bufs=1) as kp,
            tc.tile_pool(name="stg", bufs=6) as stg,
            tc.tile_pool(name="xs", bufs=2) as xs,
            tc.tile_pool(name="kvs", bufs=2) as kvs,
            tc.tile_pool(name="att", bufs=6) as ap,
            tc.tile_pool(name="att_out", bufs=3) as op,
            tc.tile_pool(name="pps", bufs=5, space="PSUM") as pps,
            tc.tile_pool(name="avs", bufs=2, space="PSUM") as avsum,
            tc.tile_pool(name="ops", bufs=1, space="PSUM") as opsum,
        ):
            ones = persist.tile([P, 16], BF, tag="ones", name="ones")
            nc.vector.memset(ones[:], 1.0)
            zbias = persist.tile([P, 1], F32, tag="zbias", name="zbias")
            nc.vector.memset(zbias[:], 0.0)
            qT = [persist.tile([P, ROWS], BF, tag=f"qT{t}", name=f"qT{t}") for t in range(OT)]
            acc = {}
            for isub in range(4):
                for ob in range(2):
                    acc[isub, ob] = persist.tile([P, 512], F32, tag=f"acc{isub}{ob}", name=f"acc{isub}{ob}")
                    nc.vector.memset(acc[isub, ob][:], 0.0)
            sums_bank = opsum.tile([P, 64], F32, tag="sums", name="sums")

            wk_t = [kp.tile([P, D], BF, tag=f"wk{d_}", name=f"wk{d_}") for d_ in range(DC)]
            wv_t = [kp.tile([P, D], BF, tag=f"wv{d_}", name=f"wv{d_}") for d_ in range(DC)]
            xk_t = {}
            xv_t = {}

            def load_x_quarter(t, k_first=False):
                for d_ in range(DC):
                    xk_t[t, d_] = xs.tile([P, 512], BF, tag=f"xk{d_}", name=f"xk{d_}")
                    nc.sync.dma_start(out=xk_t[t, d_][:], in_=xk[d_ * P:(d_ + 1) * P, t * 512:(t + 1) * 512])
                    if not k_first:
                        xv_t[t, d_] = xs.tile([P, 512], BF, tag=f"xv{d_}", name=f"xv{d_}")
                        nc.sync.dma_start(out=xv_t[t, d_][:], in_=xv[d_ * P:(d_ + 1) * P, t * 512:(t + 1) * 512])
                if k_first:
                    for d_ in range(DC):
                        xv_t[t, d_] = xs.tile([P, 512], BF, tag=f"xv{d_}", name=f"xv{d_}")
                        nc.sync.dma_start(out=xv_t[t, d_][:], in_=xv[d_ * P:(d_ + 1) * P, t * 512:(t + 1) * 512])

            def kv_quarter(t):
                for ohi in range(OT):
                    ps = pps.tile([P, 512], F32, tag="pp", name="ppk")
                    for d_ in range(DC):
                        nc.tensor.matmul(
                            ps[:],
                            lhsT=wk_t[d_][:, ohi * P:(ohi + 1) * P],
                            rhs=xk_t[t, d_][:],
                            start=(d_ == 0),
                            stop=(d_ == DC - 1),
                        )
                    sg = stg.tile([P, 512], BF, tag="sg", name="sg")
                    nc.scalar.copy(sg[:], ps[:])
                    nc.gpsimd.dma_start(out=kvin[t][:, ohi, :], in_=sg[:])
                for jh in range(4):
                    for ob in range(2):
                        ps = pps.tile([P, 512], F32, tag="pp", name="ppv")
                        for d_ in range(DC):
                            nc.tensor.matmul(
                                ps[:],
                                lhsT=xv_t[t, d_][:, jh * P:(jh + 1) * P],
                                rhs=wv_t[d_][:, ob * 512:(ob + 1) * 512],
                                start=(d_ == 0),
                                stop=(d_ == DC - 1),
                            )
                        sg = stg.tile([P, 512], BF, tag="sg", name="sg")
                        nc.scalar.copy(sg[:], ps[:])
                        nc.gpsimd.dma_start(out=kvin[t][:, 8 + ob * 4 + jh, :], in_=sg[:])
                nc.gpsimd.collective_compute(
                    "AllGather",
                    mybir.AluOpType.bypass,
                    replica_groups=PAIRS,
                    ins=[kvin[t][:].opt()],
                    outs=[kvout[t][:].opt()],
                )

            def attn_quarter(qtr):
                t, g = qtr // 2, qtr % 2
                ktq = kvs.tile([P, OT, 512], BF, tag="ktq", name="ktq")
                nc.scalar.dma_start(out=ktq[:], in_=kvout[t][g * P:(g + 1) * P, 0:8, :])
                vtq = kvs.tile([P, OT, 512], BF, tag="vtq", name="vtq")
                nc.scalar.dma_start(out=vtq[:], in_=kvout[t][g * P:(g + 1) * P, 8:16, :])
                ptq = []
                for jl in range(4):
                    jt = qtr * 4 + jl
                    sp = pps.tile([P, ROWS], F32, tag="pp", name="sps")
                    for oc in range(OT):
                        nc.tensor.matmul(
                            sp[:],
                            lhsT=ktq[:, oc, jl * P:(jl + 1) * P],
                            rhs=qT[oc][:],
                            start=(oc == 0),
                            stop=(oc == OT - 1),
                        )
                    pt = ap.tile([P, ROWS], BF, tag="pt", name="pt")
                    nc.scalar.activation(pt[:], sp[:], EXP, bias=zbias[:])
                    mt = ap.tile([P, ROWS], BF, tag="mt", name="mt")
                    nc.gpsimd.dma_start(out=mt[:], in_=msk[jt, :, :])
                    nc.vector.tensor_mul(pt[:], pt[:], mt[:])
                    ptq.append(pt)
                    for isub in range(4):
                        nc.tensor.matmul(
                            sums_bank[:, isub * 16:(isub + 1) * 16],
                            lhsT=pt[:, isub * P:(isub + 1) * P],
                            rhs=ones[:],
                            start=(jt == 0 and isub == 0),
                            stop=(jt == NJT - 1 and isub == 3),
                            skip_group_check=True,
                        )
                for isub in range(4):
                    for ob in range(2):
                        sc = avsum.tile([P, 512], F32, tag="avs", name="avs")
                        for jl in range(4):
                            nc.tensor.matmul(
                                sc[:],
                                lhsT=ptq[jl][:, isub * P:(isub + 1) * P],
                                rhs=vtq[:, ob * 4 + jl, :],
                                start=(jl == 0),
                                stop=(jl == 3),
                            )
                        nc.vector.tensor_add(acc[isub, ob][:], acc[isub, ob][:], sc[:])

            # ---- interleaved schedule: K weights + x_k q0 load first ----
            for d_ in range(DC):
                nc.sync.dma_start(out=wk_t[d_][:], in_=wk[d_ * P:(d_ + 1) * P, :])
            load_x_quarter(0, k_first=True)
            for d_ in range(DC):
                nc.sync.dma_start(out=wv_t[d_][:], in_=wv[d_ * P:(d_ + 1) * P, :])
            kv_quarter(0)
            load_x_quarter(1)
            kv_quarter(1)

            xq_t = [kp.tile([P, ROWS], BF, tag=f"xq{d_}", name=f"xq{d_}") for d_ in range(DC)]
            wq_t = [kp.tile([P, D], BF, tag=f"wq{d_}", name=f"wq{d_}") for d_ in range(DC)]
            for d_ in range(DC):
                nc.sync.dma_start(out=xq_t[d_][:], in_=xq[d_ * P:(d_ + 1) * P, :])
                nc.sync.dma_start(out=wq_t[d_][:], in_=wq[d_ * P:(d_ + 1) * P, :])
            for t in range(OT):
                ps = pps.tile([P, ROWS], F32, tag="pp", name="ppq")
                for d_ in range(DC):
                    nc.tensor.matmul(
                        ps[:],
                        lhsT=wq_t[d_][:, t * P:(t + 1) * P],
                        rhs=xq_t[d_][:],
                        start=(d_ == 0),
                        stop=(d_ == DC - 1),
                    )
                nc.vector.tensor_copy(qT[t][:], ps[:])

            load_x_quarter(2)
            kv_quarter(2)
            attn_quarter(0)
            load_x_quarter(3)
            kv_quarter(3)
            for qtr in range(1, 8):
                attn_quarter(qtr)

            for isub in range(4):
                ssb = op.tile([P, 1], F32, tag="ssb", name="ssb")
                nc.vector.tensor_copy(ssb[:], sums_bank[:, isub * 16:isub * 16 + 1])
                rec = op.tile([P, 1], F32, tag=f"rec{isub}", name=f"rec{isub}")
                nc.vector.reciprocal(rec[:], ssb[:])
                for ob in range(2):
                    osb = op.tile([P, 512], F32, tag="osb", name="osb")
                    nc.vector.tensor_scalar_mul(osb[:], acc[isub, ob][:], rec[:])
                    nc.sync.dma_start(out=out[isub * P:(isub + 1) * P, ob * 512:(ob + 1) * 512], in_=osb[:])
    return nc


_CACHE = {}


def _get_nc():
    if "nc" not in _CACHE:
        nc = build_nc()
        nc.compile()
        _CACHE["nc"] = nc
    return _CACHE["nc"]


def build_in_maps(inputs):
    x_q = np.asarray(inputs["encodings_for_q"], dtype=np.float32)
    x_k = np.asarray(inputs["encodings_for_k"], dtype=np.float32)
    x_v = np.asarray(inputs["encodings_for_v"], dtype=np.float32)
    W_q = np.asarray(inputs["W_q"], dtype=np.float32)
    W_k = np.asarray(inputs["W_k"], dtype=np.float32)
    W_v = np.asarray(inputs["W_v"], dtype=np.float32)

    wqt = np.ascontiguousarray(W_q.T).astype(bf16)
    wkt = np.ascontiguousarray(W_k.T / np.sqrt(D)).astype(bf16)
    wvt = np.ascontiguousarray(W_v.T).astype(bf16)

    causal = (np.arange(S)[:, None] <= np.arange(S)[None, :])

    in_maps = []
    for c in range(NCORES):
        rows = slice(ROWS * c, ROWS * (c + 1))
        h = slice(HALF * (c % 2), HALF * (c % 2 + 1))
        xqt_c = np.ascontiguousarray(x_q[rows].T).astype(bf16)
        xkh_c = np.ascontiguousarray(x_k[h].T).astype(bf16)
        xvh_c = np.ascontiguousarray(x_v[h].T).astype(bf16)
        m = causal[:, rows]
        mg = m.reshape(NJT, P, ROWS)
        order = []
        for qtr in range(8):
            t, g = qtr // 2, qtr % 2
            for jl in range(4):
                order.append(16 * g + 4 * t + jl)
        mask_c = np.ascontiguousarray(mg[order]).astype(bf16)
        in_maps.append(
            dict(
                xqt=xqt_c, xkh=xkh_c, xvh=xvh_c,
                wqt=wqt, wkt=wkt, wvt=wvt,
                mask01=mask_c,
            )
        )
    return in_maps


def kernel(**inputs):
    nc = _get_nc()
    in_maps = build_in_maps(inputs)
    res = run_bass_kernel_spmd(nc, in_maps, list(range(NCORES)))
    outs = [np.asarray(res.results[i]["out"], dtype=np.float32) for i in range(NCORES)]
    return np.concatenate(outs, axis=0)



# revision 17
# speedup vs baseline: 1.3809x; 1.3809x over previous
"""Causal single-head attention (S=4096, D=1024, fp32) on 8 TRN2 NeuronCores.

v7: causal-aware fold-balanced schedule, 8-way-sharded K/V projection with
fp8 transport, DoubleRow fp8 scores.

Row ownership (fold): core c owns row blocks c and 15-c (256 rows each),
packed as qT columns [top | bot]. Every core needs key block k for its top
iff k <= c and for its bot iff k <= 15-c, so the uniform SPMD program
processes key blocks 0-7 against all 512 rows and blocks 8-15 against the
bot 256 rows only; per-core causal variation is applied via small 0/1 mask
tiles multiplied into p (3MB/core, vs 4MB in the dense baseline -- and the
score matmuls now cover 24/64 of the dense rectangle instead of all of it).

K/V projection is sharded 8-way: core c computes K^T and V for key blocks
c (gather A) and 8+c (gather B) in bf16, restages to fp8, and two 8-way
AllGathers (4MB out each) distribute them. fp8 halves collective bytes --
the CC stream is otherwise the critical path. V for blocks 0,1 is also
computed locally in bf16 on every core because early rows average too few
keys to tolerate fp8 V noise (row 0's output IS one v row).

Scores run as DoubleRow fp8 matmuls (q,k fp8: 2x PE throughput); p stays
bf16 (fp8 p fails precision); A@V runs bf16 x fp8. exp uses bias -2 so a
later fp8-p experiment stays in e4m3 normal range; the shift cancels in
softmax. 1/sqrt(D) is folded as D**-0.25 into BOTH W_q and W_k host-side
so fp8 q/k stay in e4m3 normal range (folding 1/32 into one side would
push that side subnormal).
"""

import numpy as np
import ml_dtypes

import concourse.bacc as bacc
import concourse.tile as tile
from concourse import mybir
from concourse.bass_utils import run_bass_kernel_spmd

S = 4096
D = 1024
NCORES = 8
P = 128
RPC = 512          # rows per core
KB = 256           # key block
NKB = 16
DC = 8             # d_in chunks of 128
BF = mybir.dt.bfloat16
F8 = mybir.dt.float8e4
F32 = mybir.dt.float32
EXP = mybir.ActivationFunctionType.Exp
DR = mybir.MatmulPerfMode.DoubleRow
USE_DR = True
DEBUG = False

bf16 = ml_dtypes.bfloat16
f8e4 = ml_dtypes.float8_e4m3fn

QK_DT = F8 if USE_DR else BF
# kvin/kvout layout per core contribution (256 keys = its key block), as 16
# sections of 256:
#   sections 0-7:  K^T, sec = 2*i + t (d-pair i, half t), offset = key
#                  (partition = d0)
#   sections 8-15: V, sec = 8 + 4*kt + 2*half + s, offset = d%256
#                  (partition = key; d = 512*half + 256*s + offset)


def build_nc():
    nc = bacc.Bacc(None, target_bir_lowering=False, debug=False)

    xq = nc.declare_dram_parameter("xqt", [D, RPC], BF, isOutput=False)
    xk = nc.declare_dram_parameter("xkt", [D, 512], BF, isOutput=False)
    xv = nc.declare_dram_parameter("xvt", [D, 512], BF, isOutput=False)
    xv01 = nc.declare_dram_parameter("xv01t", [D, 512], BF, isOutput=False)
    wq = nc.declare_dram_parameter("wqt", [D, D], BF, isOutput=False)
    wk = nc.declare_dram_parameter("wkt", [D, D], BF, isOutput=False)
    wv = nc.declare_dram_parameter("wvt", [D, D], BF, isOutput=False)
    mlo = nc.declare_dram_parameter("mlo", [8, P, 2, 512], BF, isOutput=False)
    mhi = nc.declare_dram_parameter("mhi", [8, P, 2, 256], BF, isOutput=False)
    out = nc.declare_dram_parameter("out", [RPC, D], F32, isOutput=True)

    kvin = [nc.dram_tensor(f"kvin{g}", [P, 16, 256], F8) for g in range(2)]
    kvout = [nc.dram_tensor(f"kvout{g}", [NCORES * P, 16, 256], F8) for g in range(2)]
    if DEBUG:
        dbg_sp = nc.declare_dram_parameter("dbg_sp", [2, P, 512], F32, isOutput=True)
        dbg_p = nc.declare_dram_parameter("dbg_p", [2, P, 2, 512], F32, isOutput=True)
        dbg_sums = nc.declare_dram_parameter("dbg_sums", [P, 64], F32, isOutput=True)

    with tile.TileContext(nc) as tc:
        with (
            tc.tile_pool(name="persist", bufs=1) as persist,
            tc.tile_pool(name="wp", bufs=1) as wp,
            tc.tile_pool(name="stg", bufs=2) as stg,
            tc.tile_pool(name="kvs", bufs=3) as kvs,
            tc.tile_pool(name="vbs", bufs=1) as vbs,
            tc.tile_pool(name="pbs", bufs=1) as pbs,
            tc.tile_pool(name="op", bufs=4) as op,
            tc.tile_pool(name="pps", bufs=3, space="PSUM") as pps,
            tc.tile_pool(name="avs", bufs=2, space="PSUM") as avs,
            tc.tile_pool(name="ops", bufs=1, space="PSUM") as ops,
        ):
            ones = persist.tile([P, 16], BF, tag="ones", name="ones")
            nc.vector.memset(ones[:], 1.0)
            nbias = persist.tile([P, 1], F32, tag="nbias", name="nbias")
            nc.vector.memset(nbias[:], -2.0)
            qT = persist.tile([P, 4, 2, RPC], QK_DT, tag="qT", name="qT")
            acc = {}
            for st in range(4):
                acc[st] = persist.tile([P, D], F32, tag=f"acc{st}", name=f"acc{st}")
                nc.vector.memset(acc[st][:], 0.0)
            vloc = [persist.tile([P, 8, 256], BF, tag=f"vloc{b}", name=f"vloc{b}") for b in range(2)]
            mlo_t = [persist.tile([P, 2, 512], BF, tag=f"mlo{k}", name=f"mlo{k}") for k in range(8)]
            mhi_t = [persist.tile([P, 2, 256], BF, tag=f"mhi{k}", name=f"mhi{k}") for k in range(8)]
            sums = ops.tile([P, 64], F32, tag="sums", name="sums")

            # ---- input loads (ordered by first use) ----
            wk_t = [wp.tile([P, D], BF, tag=f"wk{d}", name=f"wk{d}") for d in range(DC)]
            wv_t = [wp.tile([P, D], BF, tag=f"wv{d}", name=f"wv{d}") for d in range(DC)]
            wq_t = [wp.tile([P, D], BF, tag=f"wq{d}", name=f"wq{d}") for d in range(DC)]
            xk_t = [wp.tile([P, 512], BF, tag=f"xk{d}", name=f"xk{d}") for d in range(DC)]
            xv_t = [wp.tile([P, 512], BF, tag=f"xv{d}", name=f"xv{d}") for d in range(DC)]
            xq_t = [wp.tile([P, RPC], BF, tag=f"xq{d}", name=f"xq{d}") for d in range(DC)]
            xv01_t = [wp.tile([P, 512], BF, tag=f"xv01{d}", name=f"xv01{d}") for d in range(DC)]
            for d in range(DC):
                r = slice(d * P, (d + 1) * P)
                nc.sync.dma_start(out=wk_t[d][:], in_=wk[r, :])
                nc.sync.dma_start(out=xk_t[d][:], in_=xk[r, :])
            for d in range(DC):
                r = slice(d * P, (d + 1) * P)
                nc.sync.dma_start(out=wv_t[d][:], in_=wv[r, :])
                nc.sync.dma_start(out=xv_t[d][:], in_=xv[r, :])
            for d in range(DC):
                r = slice(d * P, (d + 1) * P)
                nc.sync.dma_start(out=xv01_t[d][:], in_=xv01[r, :])
                nc.sync.dma_start(out=wq_t[d][:], in_=wq[r, :])
                nc.sync.dma_start(out=xq_t[d][:], in_=xq[r, :])
            for k in range(8):
                nc.gpsimd.dma_start(out=mlo_t[k][:], in_=mlo[k, :, :, :])
            for k in range(8):
                nc.gpsimd.dma_start(out=mhi_t[k][:], in_=mhi[k, :, :, :])

            def kv_share(g):
                # project this core's 256-key share (xk/xv cols [256g, 256g+256))
                # into the fp8 staging layout and AllGather it.
                sg = stg.tile([P, 16, 256], F8, tag="sg", name="sg")
                cols = slice(256 * g, 256 * g + 256)
                for ohi in range(DC):
                    ps = pps.tile([P, 512], F32, tag="pp", name="ppk")
                    for d in range(DC):
                        nc.tensor.matmul(
                            ps[:, 0:256],
                            lhsT=wk_t[d][:, ohi * P:(ohi + 1) * P],
                            rhs=xk_t[d][:, cols],
                            start=(d == 0),
                            stop=(d == DC - 1),
                        )
                    nc.scalar.copy(sg[:, ohi, :], ps[:, 0:256])
                for kt in range(2):
                    for half in range(2):
                        ps = pps.tile([P, 512], F32, tag="pp", name="ppv")
                        for d in range(DC):
                            nc.tensor.matmul(
                                ps[:],
                                lhsT=xv_t[d][:, 256 * g + kt * P:256 * g + (kt + 1) * P],
                                rhs=wv_t[d][:, half * 512:(half + 1) * 512],
                                start=(d == 0),
                                stop=(d == DC - 1),
                            )
                        for s in range(2):
                            nc.scalar.copy(
                                sg[:, 8 + 4 * kt + 2 * half + s, :],
                                ps[:, s * 256:(s + 1) * 256],
                            )
                nc.gpsimd.dma_start(out=kvin[g][:], in_=sg[:])
                nc.gpsimd.collective_compute(
                    "AllGather",
                    mybir.AluOpType.bypass,
                    replica_groups=[[0, 1, 2, 3, 4, 5, 6, 7]],
                    ins=[kvin[g][:].opt()],
                    outs=[kvout[g][:].opt()],
                )

            kv_share(0)
            kv_share(1)

            # local bf16 V for key blocks 0,1 (fp8 V too lossy for early rows)
            for b in range(2):
                for kt in range(2):
                    for half in range(2):
                        ps = pps.tile([P, 512], F32, tag="pp", name="ppl")
                        for d in range(DC):
                            nc.tensor.matmul(
                                ps[:],
                                lhsT=xv01_t[d][:, 256 * b + kt * P:256 * b + (kt + 1) * P],
                                rhs=wv_t[d][:, half * 512:(half + 1) * 512],
                                start=(d == 0),
                                stop=(d == DC - 1),
                            )
                        for s in range(2):
                            nc.scalar.copy(
                                vloc[b][:, 4 * kt + 2 * half + s, :],
                                ps[:, s * 256:(s + 1) * 256],
                            )

            # q projection -> qT fp8 [d0, pair, t, row]
            for ohi in range(DC):
                ps = pps.tile([P, 512], F32, tag="pp", name="ppq")
                for d in range(DC):
                    nc.tensor.matmul(
                        ps[:],
                        lhsT=wq_t[d][:, ohi * P:(ohi + 1) * P],
                        rhs=xq_t[d][:],
                        start=(d == 0),
                        stop=(d == DC - 1),
                    )
                nc.scalar.copy(qT[:, ohi // 2, ohi % 2, :], ps[:])

            # ---- attention: superblock sb over gathered key blocks ----
            def attn_block(sb, b8):
                blk = 8 * sb + b8
                W = 512 if sb == 0 else 256
                roff = 0 if sb == 0 else 256
                kblk = kvs.tile([P, 8, 256], F8, tag="kb", name="kb")
                nc.scalar.dma_start(out=kblk[:], in_=kvout[sb][b8 * P:(b8 + 1) * P, 0:8, :])
                if sb == 0 and b8 < 2:
                    vblk = vloc[b8]
                else:
                    vblk = vbs.tile([P, 8, 256], F8, tag=f"vb{b8}", name=f"vb{b8}")
                    nc.scalar.dma_start(out=vblk[:], in_=kvout[sb][b8 * P:(b8 + 1) * P, 8:16, :])
                pblk = pbs.tile([P, 2, 512], BF, tag=f"pb{b8}", name=f"pb{b8}")
                mt = mlo_t[b8] if sb == 0 else mhi_t[b8]
                for kt in range(2):
                    sp = pps.tile([P, 512], F32, tag="pp", name="sp")
                    if USE_DR:
                        for i in range(4):
                            nc.tensor.matmul(
                                sp[:, 0:W],
                                lhsT=kblk[:, 2 * i:2 * i + 2, kt * P:(kt + 1) * P],
                                rhs=qT[:, i, :, roff:roff + W],
                                start=(i == 0),
                                stop=(i == 3),
                                perf_mode=DR,
                            )
                    else:
                        for i in range(4):
                            for t in range(2):
                                nc.tensor.matmul(
                                    sp[:, 0:W],
                                    lhsT=kblk[:, 2 * i + t, kt * P:(kt + 1) * P],
                                    rhs=qT[:, i, t, roff:roff + W],
                                    start=(i == 0 and t == 0),
                                    stop=(i == 3 and t == 1),
                                )
                    if DEBUG and sb == 0 and b8 == 0:
                        dsp = persist.tile([P, 512], F32, tag="dsp", name="dsp")
                        nc.vector.tensor_copy(dsp[:], sp[:])
                        nc.sync.dma_start(out=dbg_sp[kt, :, :], in_=dsp[:])
                    nc.scalar.activation(pblk[:, kt, 0:W], sp[:, 0:W], EXP, bias=nbias[:])
                    nc.vector.tensor_mul(pblk[:, kt, 0:W], pblk[:, kt, 0:W], mt[:, kt, 0:W])
                    if DEBUG and sb == 0 and b8 == 0:
                        dp = persist.tile([P, 512], F32, tag="dp", name="dp")
                        nc.vector.tensor_copy(dp[:], pblk[:, kt, :])
                        nc.sync.dma_start(out=dbg_p[kt, :, 0, :], in_=dp[:])
                        dm = persist.tile([P, 512], F32, tag="dm", name="dm")
                        nc.vector.tensor_copy(dm[:], mt[:, kt, :])
                        nc.sync.dma_start(out=dbg_p[kt, :, 1, :], in_=dm[:])
                    # denominator partial sums: region stl covers global row
                    # subtile; lo regions close at end of sb0, hi at end of sb1.
                    # start=True clears the WHOLE psum bank, so it may only be
                    # set on the very first matmul into the sums bank; cleared
                    # elements overwrite-on-first-touch via has_written bits.
                    for stl in range(W // P):
                        stg_ = stl if sb == 0 else stl + 2
                        first = blk == 0 and kt == 0 and stl == 0
                        last = sb == 1 and b8 == 7 and kt == 1 and stl == 1
                        nc.tensor.matmul(
                            sums[:, stg_ * 16:(stg_ + 1) * 16],
                            lhsT=pblk[:, kt, stl * P:(stl + 1) * P],
                            rhs=ones[:],
                            start=first,
                            stop=last,
                            skip_group_check=True,
                        )
                return pblk, vblk

            def attn_av(sb, tiles):
                sts = (0, 1, 2, 3) if sb == 0 else (2, 3)
                roff = 0 if sb == 0 else 256
                for st in sts:
                    stl = st * P - roff
                    for half in range(2):
                        av = avs.tile([P, 512], F32, tag="av", name="av")
                        n = len(tiles)
                        for j, (pblk, vblk) in enumerate(tiles):
                            for kt in range(2):
                                nc.tensor.matmul(
                                    av[:],
                                    lhsT=pblk[:, kt, stl:stl + P],
                                    rhs=vblk[:, 4 * kt + 2 * half:4 * kt + 2 * half + 2, :],
                                    start=(j == 0 and kt == 0),
                                    stop=(j == n - 1 and kt == 1),
                                )
                        nc.vector.tensor_add(
                            acc[st][:, half * 512:(half + 1) * 512],
                            acc[st][:, half * 512:(half + 1) * 512],
                            av[:],
                        )

            for sb in range(2):
                tiles = [attn_block(sb, b8) for b8 in range(8)]
                attn_av(sb, tiles)

            # ---- finalize: out = acc / sums ----
            if DEBUG:
                dsm = persist.tile([P, 64], F32, tag="dsm", name="dsm")
                nc.vector.tensor_copy(dsm[:], sums[:])
                nc.sync.dma_start(out=dbg_sums[:], in_=dsm[:])
            for st in range(4):
                ssb = op.tile([P, 1], F32, tag="ssb", name="ssb")
                nc.vector.tensor_copy(ssb[:], sums[:, st * 16:st * 16 + 1])
                rec = op.tile([P, 1], F32, tag=f"rec{st}", name=f"rec{st}")
                nc.vector.reciprocal(rec[:], ssb[:])
                for half in range(2):
                    osb = op.tile([P, 512], F32, tag="osb", name="osb")
                    nc.vector.tensor_scalar_mul(osb[:], acc[st][:, half * 512:(half + 1) * 512], rec[:])
                    nc.sync.dma_start(out=out[st * P:(st + 1) * P, half * 512:(half + 1) * 512], in_=osb[:])
    return nc


_CACHE = {}


def _get_nc():
    if "nc" not in _CACHE:
        nc = build_nc()
        nc.compile()
        _CACHE["nc"] = nc
    return _CACHE["nc"]


def build_in_maps(inputs):
    x_q = np.asarray(inputs["encodings_for_q"], dtype=np.float32)
    x_k = np.asarray(inputs["encodings_for_k"], dtype=np.float32)
    x_v = np.asarray(inputs["encodings_for_v"], dtype=np.float32)
    W_q = np.asarray(inputs["W_q"], dtype=np.float32)
    W_k = np.asarray(inputs["W_k"], dtype=np.float32)
    W_v = np.asarray(inputs["W_v"], dtype=np.float32)

    qs = D ** -0.25
    wqt = np.ascontiguousarray(W_q.T * qs).astype(bf16)
    wkt = np.ascontiguousarray(W_k.T * qs).astype(bf16)
    wvt = np.ascontiguousarray(W_v.T).astype(bf16)
    xv01t = np.ascontiguousarray(x_v[0:512].T).astype(bf16)

    in_maps = []
    for c in range(NCORES):
        top = slice(KB * c, KB * (c + 1))
        bot = slice(KB * (15 - c), KB * (16 - c))
        xqt = np.ascontiguousarray(
            np.concatenate([x_q[top], x_q[bot]], axis=0).T).astype(bf16)
        ksel = np.concatenate([x_k[top], x_k[KB * (8 + c):KB * (9 + c)]], axis=0)
        vsel = np.concatenate([x_v[top], x_v[KB * (8 + c):KB * (9 + c)]], axis=0)
        xkt = np.ascontiguousarray(ksel.T).astype(bf16)
        xvt = np.ascontiguousarray(vsel.T).astype(bf16)

        # masks: rows_global[j] for the packed qT columns
        rows = np.concatenate([np.arange(KB * c, KB * (c + 1)),
                               np.arange(KB * (15 - c), KB * (16 - c))])
        p_idx = np.arange(P)
        mlo = np.zeros((8, P, 2, 512), dtype=np.float32)
        mhi = np.zeros((8, P, 2, 256), dtype=np.float32)
        for k in range(8):
            for t in range(2):
                keys = KB * k + P * t + p_idx
                mlo[k, :, t, :] = (rows[None, :] >= keys[:, None])
                keys_h = 2048 + KB * k + P * t + p_idx
                mhi[k, :, t, :] = (rows[None, 256:] >= keys_h[:, None])
        in_maps.append(
            dict(
                xqt=xqt, xkt=xkt, xvt=xvt, xv01t=xv01t,
                wqt=wqt, wkt=wkt, wvt=wvt,
                mlo=mlo.astype(bf16), mhi=mhi.astype(bf16),
            )
        )
    return in_maps


def kernel(**inputs):
    nc = _get_nc()
    in_maps = build_in_maps(inputs)
    res = run_bass_kernel_spmd(nc, in_maps, list(range(NCORES)))
    outs = [np.asarray(res.results[i]["out"], dtype=np.float32) for i in range(NCORES)]
    full = np.empty((S, D), dtype=np.float32)
    for c in range(NCORES):
        full[KB * c:KB * (c + 1)] = outs[c][0:KB]
        full[KB * (15 - c):KB * (16 - c)] = outs[c][KB:2 * KB]
    return full


# revision 27
# speedup vs baseline: 1.4378x; 1.0411x over previous
"""Causal single-head attention (S=4096, D=1024, fp32) on 8 TRN2 NeuronCores.

v7: causal-aware fold-balanced schedule, 8-way-sharded K/V projection with
fp8 transport, DoubleRow fp8 scores.

Row ownership (fold): core c owns row blocks c and 15-c (256 rows each),
packed as qT columns [top | bot]. Every core needs key block k for its top
iff k <= c and for its bot iff k <= 15-c, so the uniform SPMD program
processes key blocks 0-7 against all 512 rows and blocks 8-15 against the
bot 256 rows only; per-core causal variation is applied via small 0/1 mask
tiles multiplied into p (3MB/core, vs 4MB in the dense baseline -- and the
score matmuls now cover 24/64 of the dense rectangle instead of all of it).

K/V projection is sharded 8-way: core c computes K^T and V for key blocks
c (gather A) and 8+c (gather B) in bf16, restages to fp8, and two 8-way
AllGathers (4MB out each) distribute them. fp8 halves collective bytes --
the CC stream is otherwise the critical path. V for blocks 0,1 is also
computed locally in bf16 on every core because early rows average too few
keys to tolerate fp8 V noise (row 0's output IS one v row).

Scores run as DoubleRow fp8 matmuls (q,k fp8: 2x PE throughput); p stays
bf16 (fp8 p fails precision); A@V runs bf16 x fp8. exp uses bias -2 so a
later fp8-p experiment stays in e4m3 normal range; the shift cancels in
softmax. 1/sqrt(D) is folded as D**-0.25 into BOTH W_q and W_k host-side
so fp8 q/k stay in e4m3 normal range (folding 1/32 into one side would
push that side subnormal).
"""

import numpy as np
import ml_dtypes

import concourse.bacc as bacc
import concourse.tile as tile
from concourse import mybir
from concourse.bass_utils import run_bass_kernel_spmd

S = 4096
D = 1024
NCORES = 8
P = 128
RPC = 512          # rows per core
KB = 256           # key block
NKB = 16
DC = 8             # d_in chunks of 128
BF = mybir.dt.bfloat16
F8 = mybir.dt.float8e4
F32 = mybir.dt.float32
EXP = mybir.ActivationFunctionType.Exp
DR = mybir.MatmulPerfMode.DoubleRow
USE_DR = True
DEBUG = False

bf16 = ml_dtypes.bfloat16
f8e4 = ml_dtypes.float8_e4m3fn

QK_DT = F8 if USE_DR else BF
# kvin/kvout layout per core contribution (256 keys = its key block), as 16
# sections of 256:
#   sections 0-7:  K^T, sec = 2*i + t (d-pair i, half t), offset = key
#                  (partition = d0)
#   sections 8-15: V, sec = 8 + 4*kt + 2*half + s, offset = d%256
#                  (partition = key; d = 512*half + 256*s + offset)


def build_nc():
    nc = bacc.Bacc(None, target_bir_lowering=False, debug=False)

    xq = nc.declare_dram_parameter("xqt", [D, RPC], BF, isOutput=False)
    xk = nc.declare_dram_parameter("xkt", [D, 512], BF, isOutput=False)
    xv = nc.declare_dram_parameter("xvt", [D, 512], BF, isOutput=False)
    xv01 = nc.declare_dram_parameter("xv01t", [D, 512], BF, isOutput=False)
    wq = nc.declare_dram_parameter("wqt", [D, D], BF, isOutput=False)
    wk = nc.declare_dram_parameter("wkt", [D, D], BF, isOutput=False)
    wv = nc.declare_dram_parameter("wvt", [D, D], BF, isOutput=False)
    mlo = nc.declare_dram_parameter("mlo", [8, P, 2, 512], BF, isOutput=False)
    mhi = nc.declare_dram_parameter("mhi", [8, P, 2, 256], BF, isOutput=False)
    out = nc.declare_dram_parameter("out", [RPC, D], F32, isOutput=True)

    # gather 1: K blocks 0-7 (scores sb0 unblocks earliest), gather 2:
    # V blocks 0-7 + K blocks 8-15, gather 3: V blocks 8-15
    kvin_k0 = nc.dram_tensor("kvin_k0", [P, 8, 256], F8)
    kvout_k0 = nc.dram_tensor("kvout_k0", [NCORES * P, 8, 256], F8)
    kvin_m = nc.dram_tensor("kvin_m", [P, 16, 256], F8)
    kvout_m = nc.dram_tensor("kvout_m", [NCORES * P, 16, 256], F8)
    kvin_v1 = nc.dram_tensor("kvin_v1", [P, 8, 256], F8)
    kvout_v1 = nc.dram_tensor("kvout_v1", [NCORES * P, 8, 256], F8)
    if DEBUG:
        dbg_sp = nc.declare_dram_parameter("dbg_sp", [2, P, 512], F32, isOutput=True)
        dbg_p = nc.declare_dram_parameter("dbg_p", [2, P, 2, 512], F32, isOutput=True)
        dbg_sums = nc.declare_dram_parameter("dbg_sums", [P, 64], F32, isOutput=True)

    with tile.TileContext(nc) as tc:
        with (
            tc.tile_pool(name="persist", bufs=1) as persist,
            tc.tile_pool(name="wp", bufs=1) as wp,
            tc.tile_pool(name="stg", bufs=1) as stg,
            tc.tile_pool(name="kvs", bufs=3) as kvs,
            tc.tile_pool(name="vbs", bufs=1) as vbs,
            tc.tile_pool(name="pbs", bufs=1) as pbs,
            tc.tile_pool(name="op", bufs=4) as op,
            tc.tile_pool(name="pps", bufs=3, space="PSUM") as pps,
            tc.tile_pool(name="avs", bufs=2, space="PSUM") as avs,
            tc.tile_pool(name="ops", bufs=1, space="PSUM") as ops,
        ):
            ones = persist.tile([P, 16], BF, tag="ones", name="ones")
            nc.vector.memset(ones[:], 1.0)
            nbias = persist.tile([P, 1], F32, tag="nbias", name="nbias")
            nc.vector.memset(nbias[:], -2.0)
            qT = persist.tile([P, 4, 2, RPC], QK_DT, tag="qT", name="qT")
            acc = {}
            for st in range(4):
                acc[st] = persist.tile([P, D], F32, tag=f"acc{st}", name=f"acc{st}")
                nc.vector.memset(acc[st][:], 0.0)
            vloc = [persist.tile([P, 8, 256], BF, tag=f"vloc{b}", name=f"vloc{b}") for b in range(2)]
            mlo_t = [persist.tile([P, 2, 512], BF, tag=f"mlo{k}", name=f"mlo{k}") for k in range(8)]
            mhi_t = [persist.tile([P, 2, 256], BF, tag=f"mhi{k}", name=f"mhi{k}") for k in range(8)]
            sums = ops.tile([P, 64], F32, tag="sums", name="sums")

            # ---- input loads (ordered by first use) ----
            wk_t = [wp.tile([P, D], BF, tag=f"wk{d}", name=f"wk{d}") for d in range(DC)]
            wv_t = [wp.tile([P, D], BF, tag=f"wv{d}", name=f"wv{d}") for d in range(DC)]
            wq_t = [wp.tile([P, D], BF, tag=f"wq{d}", name=f"wq{d}") for d in range(DC)]
            xk_t = [wp.tile([P, 512], BF, tag=f"xk{d}", name=f"xk{d}") for d in range(DC)]
            xv_t = [wp.tile([P, 512], BF, tag=f"xv{d}", name=f"xv{d}") for d in range(DC)]
            xq_t = [wp.tile([P, RPC], BF, tag=f"xq{d}", name=f"xq{d}") for d in range(DC)]
            xv01_t = [wp.tile([P, 512], BF, tag=f"xv01{d}", name=f"xv01{d}") for d in range(DC)]
            for d in range(DC):
                r = slice(d * P, (d + 1) * P)
                nc.sync.dma_start(out=wk_t[d][:], in_=wk[r, :])
                nc.sync.dma_start(out=xk_t[d][:], in_=xk[r, :])
            for d in range(DC):
                r = slice(d * P, (d + 1) * P)
                nc.sync.dma_start(out=wv_t[d][:], in_=wv[r, :])
                nc.sync.dma_start(out=xv_t[d][:], in_=xv[r, :])
            for d in range(DC):
                r = slice(d * P, (d + 1) * P)
                nc.sync.dma_start(out=xv01_t[d][:], in_=xv01[r, :])
                nc.sync.dma_start(out=wq_t[d][:], in_=wq[r, :])
                nc.sync.dma_start(out=xq_t[d][:], in_=xq[r, :])
            # masks go on the sync queue: gpsimd must reach the collective
            # triggers as early as possible (it blocks on each AG completion)
            for k in range(8):
                nc.sync.dma_start(out=mlo_t[k][:], in_=mlo[k, :, :, :])
            for k in range(8):
                nc.sync.dma_start(out=mhi_t[k][:], in_=mhi[k, :, :, :])

            def stage_k(g, sg, secoff):
                # project K^T for this core's 256-key share g into fp8 secs
                cols = slice(256 * g, 256 * g + 256)
                for ohi in range(DC):
                    ps = pps.tile([P, 512], F32, tag="pp", name="ppk")
                    for d in range(DC):
                        nc.tensor.matmul(
                            ps[:, 0:256],
                            lhsT=wk_t[d][:, ohi * P:(ohi + 1) * P],
                            rhs=xk_t[d][:, cols],
                            start=(d == 0),
                            stop=(d == DC - 1),
                        )
                    nc.scalar.copy(sg[:, secoff + ohi, :], ps[:, 0:256])

            def stage_v(g, sg, secoff):
                for kt in range(2):
                    for half in range(2):
                        ps = pps.tile([P, 512], F32, tag="pp", name="ppv")
                        for d in range(DC):
                            nc.tensor.matmul(
                                ps[:],
                                lhsT=xv_t[d][:, 256 * g + kt * P:256 * g + (kt + 1) * P],
                                rhs=wv_t[d][:, half * 512:(half + 1) * 512],
                                start=(d == 0),
                                stop=(d == DC - 1),
                            )
                        for s in range(2):
                            nc.scalar.copy(
                                sg[:, secoff + 4 * kt + 2 * half + s, :],
                                ps[:, s * 256:(s + 1) * 256],
                            )

            def gather(kvi, kvo):
                nc.gpsimd.collective_compute(
                    "AllGather",
                    mybir.AluOpType.bypass,
                    replica_groups=[[0, 1, 2, 3, 4, 5, 6, 7]],
                    ins=[kvi[:].opt()],
                    outs=[kvo[:].opt()],
                )

            # kvin staging DMAs go on scalar so the gpsimd queue reaches the
            # collective doorbells as early as possible
            sg1 = stg.tile([P, 8, 256], F8, tag="sg1", name="sg1")
            stage_k(0, sg1, 0)
            nc.scalar.dma_start(out=kvin_k0[:], in_=sg1[:])
            gather(kvin_k0, kvout_k0)
            sg2 = stg.tile([P, 16, 256], F8, tag="sg2", name="sg2")
            stage_v(0, sg2, 0)
            stage_k(1, sg2, 8)
            nc.scalar.dma_start(out=kvin_m[:], in_=sg2[:])
            gather(kvin_m, kvout_m)
            sg3 = stg.tile([P, 8, 256], F8, tag="sg3", name="sg3")
            stage_v(1, sg3, 0)
            nc.scalar.dma_start(out=kvin_v1[:], in_=sg3[:])
            gather(kvin_v1, kvout_v1)

            # local bf16 V for key blocks 0,1 (fp8 V too lossy for early rows)
            for b in range(2):
                for kt in range(2):
                    for half in range(2):
                        ps = pps.tile([P, 512], F32, tag="pp", name="ppl")
                        for d in range(DC):
                            nc.tensor.matmul(
                                ps[:],
                                lhsT=xv01_t[d][:, 256 * b + kt * P:256 * b + (kt + 1) * P],
                                rhs=wv_t[d][:, half * 512:(half + 1) * 512],
                                start=(d == 0),
                                stop=(d == DC - 1),
                            )
                        for s in range(2):
                            nc.scalar.copy(
                                vloc[b][:, 4 * kt + 2 * half + s, :],
                                ps[:, s * 256:(s + 1) * 256],
                            )

            # q projection -> qT fp8 [d0, pair, t, row]
            for ohi in range(DC):
                ps = pps.tile([P, 512], F32, tag="pp", name="ppq")
                for d in range(DC):
                    nc.tensor.matmul(
                        ps[:],
                        lhsT=wq_t[d][:, ohi * P:(ohi + 1) * P],
                        rhs=xq_t[d][:],
                        start=(d == 0),
                        stop=(d == DC - 1),
                    )
                nc.scalar.copy(qT[:, ohi // 2, ohi % 2, :], ps[:])

            # ---- attention: superblock sb over gathered key blocks ----
            def attn_block(sb, b8):
                blk = 8 * sb + b8
                W = 512 if sb == 0 else 256
                roff = 0 if sb == 0 else 256
                # K/V block loads on sync queue — keeping them off the scalar
                # queue avoids head-of-line blocking of exp behind a DMA that
                # waits for a late AllGather
                rows = slice(b8 * P, (b8 + 1) * P)
                kblk = kvs.tile([P, 8, 256], F8, tag="kb", name="kb")
                if sb == 0:
                    nc.sync.dma_start(out=kblk[:], in_=kvout_k0[rows, 0:8, :])
                else:
                    nc.sync.dma_start(out=kblk[:], in_=kvout_m[rows, 8:16, :])
                # vblk waits a later gather than kblk: gpsimd queue (idle after
                # the doorbells) so it can't head-of-line block kblk loads
                if sb == 0 and b8 < 2:
                    vblk = vloc[b8]
                else:
                    vblk = vbs.tile([P, 8, 256], F8, tag=f"vb{b8}", name=f"vb{b8}")
                    if sb == 0:
                        nc.gpsimd.dma_start(out=vblk[:], in_=kvout_m[rows, 0:8, :])
                    else:
                        nc.gpsimd.dma_start(out=vblk[:], in_=kvout_v1[rows, 0:8, :])
                pblk = pbs.tile([P, 2, 512], BF, tag=f"pb{b8}", name=f"pb{b8}")
                mt = mlo_t[b8] if sb == 0 else mhi_t[b8]
                for kt in range(2):
                    sp = pps.tile([P, 512], F32, tag="pp", name="sp")
                    if USE_DR:
                        for i in range(4):
                            nc.tensor.matmul(
                                sp[:, 0:W],
                                lhsT=kblk[:, 2 * i:2 * i + 2, kt * P:(kt + 1) * P],
                                rhs=qT[:, i, :, roff:roff + W],
                                start=(i == 0),
                                stop=(i == 3),
                                perf_mode=DR,
                            )
                    else:
                        for i in range(4):
                            for t in range(2):
                                nc.tensor.matmul(
                                    sp[:, 0:W],
                                    lhsT=kblk[:, 2 * i + t, kt * P:(kt + 1) * P],
                                    rhs=qT[:, i, t, roff:roff + W],
                                    start=(i == 0 and t == 0),
                                    stop=(i == 3 and t == 1),
                                )
                    if DEBUG and sb == 0 and b8 == 0:
                        dsp = persist.tile([P, 512], F32, tag="dsp", name="dsp")
                        nc.vector.tensor_copy(dsp[:], sp[:])
                        nc.sync.dma_start(out=dbg_sp[kt, :, :], in_=dsp[:])
                    nc.scalar.activation(pblk[:, kt, 0:W], sp[:, 0:W], EXP, bias=nbias[:])
                    nc.vector.tensor_mul(pblk[:, kt, 0:W], pblk[:, kt, 0:W], mt[:, kt, 0:W])
                    if DEBUG and sb == 0 and b8 == 0:
                        dp = persist.tile([P, 512], F32, tag="dp", name="dp")
                        nc.vector.tensor_copy(dp[:], pblk[:, kt, :])
                        nc.sync.dma_start(out=dbg_p[kt, :, 0, :], in_=dp[:])
                        dm = persist.tile([P, 512], F32, tag="dm", name="dm")
                        nc.vector.tensor_copy(dm[:], mt[:, kt, :])
                        nc.sync.dma_start(out=dbg_p[kt, :, 1, :], in_=dm[:])
                    # denominator partial sums: region stl covers global row
                    # subtile; lo regions close at end of sb0, hi at end of sb1.
                    # start=True clears the WHOLE psum bank, so it may only be
                    # set on the very first matmul into the sums bank; cleared
                    # elements overwrite-on-first-touch via has_written bits.
                    # Each region stops at its own last matmul so regions 0,1
                    # can be finalized while sb1 still runs.
                    for stl in range(W // P):
                        stg_ = stl if sb == 0 else stl + 2
                        first = blk == 0 and kt == 0 and stl == 0
                        last = b8 == 7 and kt == 1 and (sb == 1 or stg_ < 2)
                        nc.tensor.matmul(
                            sums[:, stg_ * 16:(stg_ + 1) * 16],
                            lhsT=pblk[:, kt, stl * P:(stl + 1) * P],
                            rhs=ones[:],
                            start=first,
                            stop=last,
                            skip_group_check=True,
                        )
                return pblk, vblk

            def attn_av(sb, tiles):
                sts = (0, 1, 2, 3) if sb == 0 else (2, 3)
                roff = 0 if sb == 0 else 256
                for st in sts:
                    stl = st * P - roff
                    for half in range(2):
                        av = avs.tile([P, 512], F32, tag="av", name="av")
                        n = len(tiles)
                        for j, (pblk, vblk) in enumerate(tiles):
                            for kt in range(2):
                                nc.tensor.matmul(
                                    av[:],
                                    lhsT=pblk[:, kt, stl:stl + P],
                                    rhs=vblk[:, 4 * kt + 2 * half:4 * kt + 2 * half + 2, :],
                                    start=(j == 0 and kt == 0),
                                    stop=(j == n - 1 and kt == 1),
                                )
                        nc.vector.tensor_add(
                            acc[st][:, half * 512:(half + 1) * 512],
                            acc[st][:, half * 512:(half + 1) * 512],
                            av[:],
                        )

            def finalize(st):
                ssb = op.tile([P, 1], F32, tag="ssb", name="ssb")
                nc.vector.tensor_copy(ssb[:], sums[:, st * 16:st * 16 + 1])
                rec = op.tile([P, 1], F32, tag=f"rec{st}", name=f"rec{st}")
                nc.vector.reciprocal(rec[:], ssb[:])
                for half in range(2):
                    osb = op.tile([P, 512], F32, tag="osb", name="osb")
                    nc.vector.tensor_scalar_mul(osb[:], acc[st][:, half * 512:(half + 1) * 512], rec[:])
                    nc.sync.dma_start(out=out[st * P:(st + 1) * P, half * 512:(half + 1) * 512], in_=osb[:])

            tiles = [attn_block(0, b8) for b8 in range(8)]
            attn_av(0, tiles)
            finalize(0)
            finalize(1)
            tiles = [attn_block(1, b8) for b8 in range(8)]
            attn_av(1, tiles)
            if DEBUG:
                dsm = persist.tile([P, 64], F32, tag="dsm", name="dsm")
                nc.vector.tensor_copy(dsm[:], sums[:])
                nc.sync.dma_start(out=dbg_sums[:], in_=dsm[:])
            finalize(2)
            finalize(3)
    return nc


_CACHE = {}


def _get_nc():
    if "nc" not in _CACHE:
        nc = build_nc()
        nc.compile()
        _CACHE["nc"] = nc
    return _CACHE["nc"]


def build_in_maps(inputs):
    x_q = np.asarray(inputs["encodings_for_q"], dtype=np.float32)
    x_k = np.asarray(inputs["encodings_for_k"], dtype=np.float32)
    x_v = np.asarray(inputs["encodings_for_v"], dtype=np.float32)
    W_q = np.asarray(inputs["W_q"], dtype=np.float32)
    W_k = np.asarray(inputs["W_k"], dtype=np.float32)
    W_v = np.asarray(inputs["W_v"], dtype=np.float32)

    qs = D ** -0.25
    wqt = np.ascontiguousarray(W_q.T * qs).astype(bf16)
    wkt = np.ascontiguousarray(W_k.T * qs).astype(bf16)
    wvt = np.ascontiguousarray(W_v.T).astype(bf16)
    xv01t = np.ascontiguousarray(x_v[0:512].T).astype(bf16)

    in_maps = []
    for c in range(NCORES):
        top = slice(KB * c, KB * (c + 1))
        bot = slice(KB * (15 - c), KB * (16 - c))
        xqt = np.ascontiguousarray(
            np.concatenate([x_q[top], x_q[bot]], axis=0).T).astype(bf16)
        ksel = np.concatenate([x_k[top], x_k[KB * (8 + c):KB * (9 + c)]], axis=0)
        vsel = np.concatenate([x_v[top], x_v[KB * (8 + c):KB * (9 + c)]], axis=0)
        xkt = np.ascontiguousarray(ksel.T).astype(bf16)
        xvt = np.ascontiguousarray(vsel.T).astype(bf16)

        # masks: rows_global[j] for the packed qT columns
        rows = np.concatenate([np.arange(KB * c, KB * (c + 1)),
                               np.arange(KB * (15 - c), KB * (16 - c))])
        p_idx = np.arange(P)
        mlo = np.zeros((8, P, 2, 512), dtype=np.float32)
        mhi = np.zeros((8, P, 2, 256), dtype=np.float32)
        for k in range(8):
            for t in range(2):
                keys = KB * k + P * t + p_idx
                mlo[k, :, t, :] = (rows[None, :] >= keys[:, None])
                keys_h = 2048 + KB * k + P * t + p_idx
                mhi[k, :, t, :] = (rows[None, 256:] >= keys_h[:, None])
        in_maps.append(
            dict(
                xqt=xqt, xkt=xkt, xvt=xvt, xv01t=xv01t,
                wqt=wqt, wkt=wkt, wvt=wvt,
                mlo=mlo.astype(bf16), mhi=mhi.astype(bf16),
            )
        )
    return in_maps


def kernel(**inputs):
    nc = _get_nc()
    in_maps = build_in_maps(inputs)
    res = run_bass_kernel_spmd(nc, in_maps, list(range(NCORES)))
    outs = [np.asarray(res.results[i]["out"], dtype=np.float32) for i in range(NCORES)]
    full = np.empty((S, D), dtype=np.float32)
    for c in range(NCORES):
        full[KB * c:KB * (c + 1)] = outs[c][0:KB]
        full[KB * (15 - c):KB * (16 - c)] = outs[c][KB:2 * KB]
    return full


# revision 28
# speedup vs baseline: 1.7638x; 1.2268x over previous
"""Causal single-head attention (S=4096, D=1024, fp32) on 8 TRN2 NeuronCores.

v9: causal fold-balanced schedule, fp8-transport sharded K/V projection,
DoubleRow fp8 scores AND A@V.

Row ownership (fold): core c owns row blocks c and 15-c (256 rows each),
packed as qT columns [top | bot]. Key block k is needed for the top half iff
k <= c and for the bot half iff k <= 15-c, so the uniform SPMD program runs
key blocks 0-7 against all 512 rows and blocks 8-15 against the bot 256
only; per-core causal variation lives in small 0/1 mask tiles multiplied
into p. Score work is 24/64 of the dense rectangle.

K/V projection is sharded 8-way (core c computes blocks c and 8+c) and
distributed by three fp8 AllGathers, ordered by when consumers need them:
  G1: K blocks 0-7   G2: V blocks 0-7 + K blocks 8-15   G3: V blocks 8-15
The first collective can't execute before a ~70-80us cross-core launch
barrier (axon environment floor), and the CC stream is serial (~90us for
8MB), so blocks 0,1 are made fully gather-free: every core computes K AND V
for key blocks 0,1 locally in bf16 and runs their scores + A@V while the
gathers are still in flight. (Local bf16 V for early blocks is also a
precision requirement: rows 0-511 average too few keys to tolerate fp8 V.)

Numerics: q,k fp8 via DoubleRow (2x PE); p is bf16 for blocks 0,1 and fp8
for blocks >= 2 (rows there average >= 513 keys, washing out fp8 p/V noise)
which enables DoubleRow A@V against fp8 V. exp uses bias -2 to keep p in
e4m3 normal range (cancels in softmax). 1/sqrt(D) is folded as D**-0.25
into BOTH W_q and W_k so fp8 q/k stay in e4m3 normal range.
"""

import numpy as np
import ml_dtypes

import concourse.bacc as bacc
import concourse.tile as tile
from concourse import mybir
from concourse.bass_utils import run_bass_kernel_spmd

S = 4096
D = 1024
NCORES = 8
P = 128
RPC = 512          # rows per core
KB = 256           # key block
DC = 8             # d_in chunks of 128
BF = mybir.dt.bfloat16
F8 = mybir.dt.float8e4
F32 = mybir.dt.float32
EXP = mybir.ActivationFunctionType.Exp
DR = mybir.MatmulPerfMode.DoubleRow
DEBUG = False

bf16 = ml_dtypes.bfloat16
f8e4 = ml_dtypes.float8_e4m3fn

# K sections: sec = ohi (d_out chunk; pairs (2i, 2i+1) feed DoubleRow),
#             offset = key within the 256-key block. Partition = d0.
# V sections: sec = 4*half + 2*kt + s, offset = d % 256
#             (d = 512*half + 256*s + offset). Partition = key within tile kt.
#             For fixed half, (kt, s, off) is contiguous -> DoubleRow rhs.


def build_nc():
    nc = bacc.Bacc(None, target_bir_lowering=False, debug=False)

    xq = nc.declare_dram_parameter("xqt", [D, RPC], BF, isOutput=False)
    xk = nc.declare_dram_parameter("xkt", [D, 512], BF, isOutput=False)
    xv = nc.declare_dram_parameter("xvt", [D, 512], BF, isOutput=False)
    xk01 = nc.declare_dram_parameter("xk01t", [D, 512], BF, isOutput=False)
    xv01 = nc.declare_dram_parameter("xv01t", [D, 512], BF, isOutput=False)
    wq = nc.declare_dram_parameter("wqt", [D, D], BF, isOutput=False)
    wk = nc.declare_dram_parameter("wkt", [D, D], BF, isOutput=False)
    wv = nc.declare_dram_parameter("wvt", [D, D], BF, isOutput=False)
    mlo = nc.declare_dram_parameter("mlo", [8, P, 2, 512], BF, isOutput=False)
    mhi = nc.declare_dram_parameter("mhi", [8, P, 2, 256], BF, isOutput=False)
    out = nc.declare_dram_parameter("out", [RPC, D], F32, isOutput=True)

    kvin_k0 = nc.dram_tensor("kvin_k0", [P, 8, 256], F8)
    kvout_k0 = nc.dram_tensor("kvout_k0", [NCORES * P, 8, 256], F8)
    kvin_m = nc.dram_tensor("kvin_m", [P, 16, 256], F8)
    kvout_m = nc.dram_tensor("kvout_m", [NCORES * P, 16, 256], F8)
    kvin_v1 = nc.dram_tensor("kvin_v1", [P, 8, 256], F8)
    kvout_v1 = nc.dram_tensor("kvout_v1", [NCORES * P, 8, 256], F8)
    if DEBUG:
        dbg_sums = nc.declare_dram_parameter("dbg_sums", [P, 64], F32, isOutput=True)

    with tile.TileContext(nc) as tc:
        with (
            tc.tile_pool(name="persist", bufs=1) as persist,
            tc.tile_pool(name="wp", bufs=1) as wp,
            tc.tile_pool(name="stg", bufs=1) as stg,
            tc.tile_pool(name="kvs", bufs=3) as kvs,
            tc.tile_pool(name="vbs", bufs=1) as vbs,
            tc.tile_pool(name="pbs", bufs=1) as pbs,
            tc.tile_pool(name="op", bufs=4) as op,
            tc.tile_pool(name="pps", bufs=3, space="PSUM") as pps,
            tc.tile_pool(name="avs", bufs=2, space="PSUM") as avs,
            tc.tile_pool(name="ops", bufs=1, space="PSUM") as ops,
        ):
            ones = persist.tile([P, 16], BF, tag="ones", name="ones")
            nc.vector.memset(ones[:], 1.0)
            nbias = persist.tile([P, 1], F32, tag="nbias", name="nbias")
            nc.vector.memset(nbias[:], -2.0)
            qT = persist.tile([P, 4, 2, RPC], F8, tag="qT", name="qT")
            acc = {}
            for st in range(4):
                acc[st] = persist.tile([P, D], F32, tag=f"acc{st}", name=f"acc{st}")
                nc.vector.memset(acc[st][:], 0.0)
            vloc = [persist.tile([P, 2, 2, 2, 256], BF, tag=f"vloc{b}", name=f"vloc{b}")
                    for b in range(2)]
            kloc = [persist.tile([P, 8, 256], F8, tag=f"kloc{b}", name=f"kloc{b}")
                    for b in range(2)]
            mlo_t = [persist.tile([P, 2, 512], BF, tag=f"mlo{k}", name=f"mlo{k}") for k in range(8)]
            mhi_t = [persist.tile([P, 2, 256], BF, tag=f"mhi{k}", name=f"mhi{k}") for k in range(8)]
            sums = ops.tile([P, 64], F32, tag="sums", name="sums")

            # ---- input loads on sync (ordered by first use) ----
            wk_t = [wp.tile([P, D], BF, tag=f"wk{d}", name=f"wk{d}") for d in range(DC)]
            wv_t = [wp.tile([P, D], BF, tag=f"wv{d}", name=f"wv{d}") for d in range(DC)]
            wq_t = [wp.tile([P, D], BF, tag=f"wq{d}", name=f"wq{d}") for d in range(DC)]
            xk_t = [wp.tile([P, 512], BF, tag=f"xk{d}", name=f"xk{d}") for d in range(DC)]
            xv_t = [wp.tile([P, 512], BF, tag=f"xv{d}", name=f"xv{d}") for d in range(DC)]
            xq_t = [wp.tile([P, RPC], BF, tag=f"xq{d}", name=f"xq{d}") for d in range(DC)]
            xk01_t = [wp.tile([P, 512], BF, tag=f"xk01{d}", name=f"xk01{d}") for d in range(DC)]
            xv01_t = [wp.tile([P, 512], BF, tag=f"xv01{d}", name=f"xv01{d}") for d in range(DC)]
            for d in range(DC):
                r = slice(d * P, (d + 1) * P)
                nc.sync.dma_start(out=wk_t[d][:], in_=wk[r, :])
                nc.sync.dma_start(out=xk_t[d][:], in_=xk[r, :])
            for d in range(DC):
                r = slice(d * P, (d + 1) * P)
                nc.sync.dma_start(out=wv_t[d][:], in_=wv[r, :])
                nc.sync.dma_start(out=xv_t[d][:], in_=xv[r, :])
            for d in range(DC):
                r = slice(d * P, (d + 1) * P)
                nc.sync.dma_start(out=xk01_t[d][:], in_=xk01[r, :])
                nc.sync.dma_start(out=xv01_t[d][:], in_=xv01[r, :])
                nc.sync.dma_start(out=wq_t[d][:], in_=wq[r, :])
                nc.sync.dma_start(out=xq_t[d][:], in_=xq[r, :])
            for k in range(8):
                nc.sync.dma_start(out=mlo_t[k][:], in_=mlo[k, :, :, :])
            for k in range(8):
                nc.sync.dma_start(out=mhi_t[k][:], in_=mhi[k, :, :, :])

            def proj_k(xt, cols, put):
                # K^T proj of 256 keys; put(ohi, psum[:, 0:256]) consumes
                for ohi in range(DC):
                    ps = pps.tile([P, 512], F32, tag="pp", name="ppk")
                    for d in range(DC):
                        nc.tensor.matmul(
                            ps[:, 0:256],
                            lhsT=wk_t[d][:, ohi * P:(ohi + 1) * P],
                            rhs=xt[d][:, cols],
                            start=(d == 0),
                            stop=(d == DC - 1),
                        )
                    put(ohi, ps)

            def proj_v(xt, base, put):
                # V proj of 256 keys; put(kt, half, s, psum[:, s*256:...])
                for kt in range(2):
                    for half in range(2):
                        ps = pps.tile([P, 512], F32, tag="pp", name="ppv")
                        for d in range(DC):
                            nc.tensor.matmul(
                                ps[:],
                                lhsT=xt[d][:, base + kt * P:base + (kt + 1) * P],
                                rhs=wv_t[d][:, half * 512:(half + 1) * 512],
                                start=(d == 0),
                                stop=(d == DC - 1),
                            )
                        for s in range(2):
                            put(kt, half, s, ps)

            def gather(kvi, kvo):
                nc.gpsimd.collective_compute(
                    "AllGather",
                    mybir.AluOpType.bypass,
                    replica_groups=[[0, 1, 2, 3, 4, 5, 6, 7]],
                    ins=[kvi[:].opt()],
                    outs=[kvo[:].opt()],
                )

            # kvin staging DMAs on scalar; doorbells (gpsimd) fire early
            sg1 = stg.tile([P, 8, 256], F8, tag="sg1", name="sg1")
            proj_k(xk_t, slice(0, 256),
                   lambda ohi, ps: nc.scalar.copy(sg1[:, ohi, :], ps[:, 0:256]))
            nc.scalar.dma_start(out=kvin_k0[:], in_=sg1[:])
            gather(kvin_k0, kvout_k0)

            sg2 = stg.tile([P, 16, 256], F8, tag="sg2", name="sg2")
            proj_v(xv_t, 0,
                   lambda kt, half, s, ps: nc.scalar.copy(
                       sg2[:, 4 * half + 2 * kt + s, :], ps[:, s * 256:(s + 1) * 256]))
            proj_k(xk_t, slice(256, 512),
                   lambda ohi, ps: nc.scalar.copy(sg2[:, 8 + ohi, :], ps[:, 0:256]))
            nc.scalar.dma_start(out=kvin_m[:], in_=sg2[:])
            gather(kvin_m, kvout_m)

            sg3 = stg.tile([P, 8, 256], F8, tag="sg3", name="sg3")
            proj_v(xv_t, 256,
                   lambda kt, half, s, ps: nc.scalar.copy(
                       sg3[:, 4 * half + 2 * kt + s, :], ps[:, s * 256:(s + 1) * 256]))
            nc.scalar.dma_start(out=kvin_v1[:], in_=sg3[:])
            gather(kvin_v1, kvout_v1)

            # local K and V for key blocks 0,1: gather-free early work
            for b in range(2):
                proj_k(xk01_t, slice(256 * b, 256 * b + 256),
                       lambda ohi, ps, b=b: nc.scalar.copy(kloc[b][:, ohi, :], ps[:, 0:256]))
                proj_v(xv01_t, 256 * b,
                       lambda kt, half, s, ps, b=b: nc.scalar.copy(
                           vloc[b][:, half, kt, s, :], ps[:, s * 256:(s + 1) * 256]))

            # q projection -> qT fp8 [d0, pair, t, row]
            for ohi in range(DC):
                ps = pps.tile([P, 512], F32, tag="pp", name="ppq")
                for d in range(DC):
                    nc.tensor.matmul(
                        ps[:],
                        lhsT=wq_t[d][:, ohi * P:(ohi + 1) * P],
                        rhs=xq_t[d][:],
                        start=(d == 0),
                        stop=(d == DC - 1),
                    )
                nc.scalar.copy(qT[:, ohi // 2, ohi % 2, :], ps[:])

            # ---- attention ----
            def attn_block(sb, b8):
                blk = 8 * sb + b8
                W = 512 if sb == 0 else 256
                roff = 0 if sb == 0 else 256
                rows = slice(b8 * P, (b8 + 1) * P)
                local = sb == 0 and b8 < 2
                if local:
                    kblk = kloc[b8]
                    vblk = vloc[b8]
                else:
                    kblk = kvs.tile([P, 8, 256], F8, tag="kb", name="kb")
                    if sb == 0:
                        nc.sync.dma_start(out=kblk[:], in_=kvout_k0[rows, 0:8, :])
                    else:
                        nc.sync.dma_start(out=kblk[:], in_=kvout_m[rows, 8:16, :])
                    # vblk on gpsimd (idle after doorbells): keeps a DMA that
                    # waits a late gather from head-of-line blocking kblk
                    vblk = vbs.tile([P, 2, 2, 2, 256], F8, tag=f"vb{b8}", name=f"vb{b8}")
                    if sb == 0:
                        nc.gpsimd.dma_start(out=vblk[:], in_=kvout_m[rows, 0:8, :])
                    else:
                        nc.gpsimd.dma_start(out=vblk[:], in_=kvout_v1[rows, 0:8, :])
                pdt = BF if local else F8
                pblk = pbs.tile([P, 2, 512], pdt, tag=f"pb{'l' if local else 'g'}{b8}",
                                name=f"pb{b8}")
                mt = mlo_t[b8] if sb == 0 else mhi_t[b8]
                for kt in range(2):
                    sp = pps.tile([P, 512], F32, tag="pp", name="sp")
                    for i in range(4):
                        nc.tensor.matmul(
                            sp[:, 0:W],
                            lhsT=kblk[:, 2 * i:2 * i + 2, kt * P:(kt + 1) * P],
                            rhs=qT[:, i, :, roff:roff + W],
                            start=(i == 0),
                            stop=(i == 3),
                            perf_mode=DR,
                        )
                    nc.scalar.activation(pblk[:, kt, 0:W], sp[:, 0:W], EXP, bias=nbias[:])
                    nc.vector.tensor_mul(pblk[:, kt, 0:W], pblk[:, kt, 0:W], mt[:, kt, 0:W])
                    # start=True clears the WHOLE psum bank: only the very
                    # first sums matmul may set it. Regions 0,1 stop at sb0's
                    # end so top rows can finalize while sb1 runs.
                    for stl in range(W // P):
                        stg_ = stl if sb == 0 else stl + 2
                        first = blk == 0 and kt == 0 and stl == 0
                        last = b8 == 7 and kt == 1 and (sb == 1 or stg_ < 2)
                        nc.tensor.matmul(
                            sums[:, stg_ * 16:(stg_ + 1) * 16],
                            lhsT=pblk[:, kt, stl * P:(stl + 1) * P],
                            rhs=ones[:],
                            start=first,
                            stop=last,
                            skip_group_check=True,
                        )
                return pblk, vblk

            def attn_av(sb, tiles, local):
                # one psum chain per (row subtile, d half) over this tile set
                sts = (0, 1, 2, 3) if sb == 0 else (2, 3)
                roff = 0 if sb == 0 else 256
                for st in sts:
                    stl = st * P - roff
                    for half in range(2):
                        av = avs.tile([P, 512], F32, tag="av", name="av")
                        n = len(tiles)
                        for j, (pblk, vblk) in enumerate(tiles):
                            if local:
                                for kt in range(2):
                                    nc.tensor.matmul(
                                        av[:],
                                        lhsT=pblk[:, kt, stl:stl + P],
                                        rhs=vblk[:, half, kt, :, :],
                                        start=(j == 0 and kt == 0),
                                        stop=(j == n - 1 and kt == 1),
                                    )
                            else:
                                nc.tensor.matmul(
                                    av[:],
                                    lhsT=pblk[:, :, stl:stl + P],
                                    rhs=vblk[:, half, :, :, :],
                                    start=(j == 0),
                                    stop=(j == n - 1),
                                    perf_mode=DR,
                                )
                        nc.vector.tensor_add(
                            acc[st][:, half * 512:(half + 1) * 512],
                            acc[st][:, half * 512:(half + 1) * 512],
                            av[:],
                        )

            def finalize(st):
                ssb = op.tile([P, 1], F32, tag="ssb", name="ssb")
                nc.vector.tensor_copy(ssb[:], sums[:, st * 16:st * 16 + 1])
                rec = op.tile([P, 1], F32, tag=f"rec{st}", name=f"rec{st}")
                nc.vector.reciprocal(rec[:], ssb[:])
                for half in range(2):
                    osb = op.tile([P, 512], F32, tag="osb", name="osb")
                    nc.vector.tensor_scalar_mul(osb[:], acc[st][:, half * 512:(half + 1) * 512], rec[:])
                    nc.sync.dma_start(out=out[st * P:(st + 1) * P, half * 512:(half + 1) * 512], in_=osb[:])

            # blocks 0,1 are fully local: scores + A@V before any gather lands
            tiles01 = [attn_block(0, b8) for b8 in range(2)]
            attn_av(0, tiles01, local=True)
            tiles27 = [attn_block(0, b8) for b8 in range(2, 8)]
            attn_av(0, tiles27, local=False)
            finalize(0)
            finalize(1)
            tiles1 = [attn_block(1, b8) for b8 in range(8)]
            attn_av(1, tiles1, local=False)
            if DEBUG:
                dsm = persist.tile([P, 64], F32, tag="dsm", name="dsm")
                nc.vector.tensor_copy(dsm[:], sums[:])
                nc.sync.dma_start(out=dbg_sums[:], in_=dsm[:])
            finalize(2)
            finalize(3)
    return nc


_CACHE = {}


def _get_nc():
    if "nc" not in _CACHE:
        nc = build_nc()
        nc.compile()
        _CACHE["nc"] = nc
    return _CACHE["nc"]


def build_in_maps(inputs):
    x_q = np.asarray(inputs["encodings_for_q"], dtype=np.float32)
    x_k = np.asarray(inputs["encodings_for_k"], dtype=np.float32)
    x_v = np.asarray(inputs["encodings_for_v"], dtype=np.float32)
    W_q = np.asarray(inputs["W_q"], dtype=np.float32)
    W_k = np.asarray(inputs["W_k"], dtype=np.float32)
    W_v = np.asarray(inputs["W_v"], dtype=np.float32)

    qs = D ** -0.25
    wqt = np.ascontiguousarray(W_q.T * qs).astype(bf16)
    wkt = np.ascontiguousarray(W_k.T * qs).astype(bf16)
    wvt = np.ascontiguousarray(W_v.T).astype(bf16)
    xk01t = np.ascontiguousarray(x_k[0:512].T).astype(bf16)
    xv01t = np.ascontiguousarray(x_v[0:512].T).astype(bf16)

    in_maps = []
    for c in range(NCORES):
        top = slice(KB * c, KB * (c + 1))
        bot = slice(KB * (15 - c), KB * (16 - c))
        xqt = np.ascontiguousarray(
            np.concatenate([x_q[top], x_q[bot]], axis=0).T).astype(bf16)
        ksel = np.concatenate([x_k[top], x_k[KB * (8 + c):KB * (9 + c)]], axis=0)
        vsel = np.concatenate([x_v[top], x_v[KB * (8 + c):KB * (9 + c)]], axis=0)
        xkt = np.ascontiguousarray(ksel.T).astype(bf16)
        xvt = np.ascontiguousarray(vsel.T).astype(bf16)

        rows = np.concatenate([np.arange(KB * c, KB * (c + 1)),
                               np.arange(KB * (15 - c), KB * (16 - c))])
        p_idx = np.arange(P)
        mlo = np.zeros((8, P, 2, 512), dtype=np.float32)
        mhi = np.zeros((8, P, 2, 256), dtype=np.float32)
        for k in range(8):
            for t in range(2):
                keys = KB * k + P * t + p_idx
                mlo[k, :, t, :] = (rows[None, :] >= keys[:, None])
                keys_h = 2048 + KB * k + P * t + p_idx
                mhi[k, :, t, :] = (rows[None, 256:] >= keys_h[:, None])
        in_maps.append(
            dict(
                xqt=xqt, xkt=xkt, xvt=xvt, xk01t=xk01t, xv01t=xv01t,
                wqt=wqt, wkt=wkt, wvt=wvt,
                mlo=mlo.astype(bf16), mhi=mhi.astype(bf16),
            )
        )
    return in_maps


def kernel(**inputs):
    nc = _get_nc()
    in_maps = build_in_maps(inputs)
    res = run_bass_kernel_spmd(nc, in_maps, list(range(NCORES)))
    outs = [np.asarray(res.results[i]["out"], dtype=np.float32) for i in range(NCORES)]
    full = np.empty((S, D), dtype=np.float32)
    for c in range(NCORES):
        full[KB * c:KB * (c + 1)] = outs[c][0:KB]
        full[KB * (15 - c):KB * (16 - c)] = outs[c][KB:2 * KB]
    return full
